# revision 2
# baseline (speedup 1.0000x reference)
"""AdaptiveGCNLayer Trainium2 kernel (8 NeuronCores, data-parallel over frames).

The reference module's adaptive-adjacency branch is dead code (its result is
never used).  Because edge_index is shared by every frame (offsets just shift
it per frame), the live computation collapses to

    out[f] = M @ x[f] @ gcn_W + gcn_b        for every frame f

with a single 25x25 normalized-adjacency matrix M (PyG GCNConv norm with
self-loops) computed on host from the 48 edges.

Device kernel (per core, 1024 frames = 25600 rows):
  - pack 5 frames = 125 rows per matmul tile
  - mm1: T1 = lhsT(x_tile).T @ (I5 (x) M^T)   -> (M5 @ X)^T in PSUM, no transposes
  - ACT copy+cast T1 -> SBUF bf16
  - mm2: O = lhsT(T1).T @ W                   -> natural row-major output in PSUM
  - DVE adds bias while copying PSUM -> SBUF
  - 1 MiB strided DMAs (f32->bf16 cast on load) stream HBM at the roofline
"""

import numpy as np
import ml_dtypes

B, V, C = 8192, 25, 128
NCORES = 8
FRAMES_PER_CORE = B // NCORES          # 1024
ROWS = FRAMES_PER_CORE * V             # 25600
FPT = 5                                # frames per matmul tile
TROWS = FPT * V                        # 125 rows per tile
NCH = 16                               # chunks (tiles) per DMA group
FULL_GROUPS = 12                       # 12 groups * 16 tiles * 125 rows = 24000 rows
# remainder: 1600 rows = 12 tiles of 125 + 1 tile of 100

_CACHE = {}


def _build_graph():
    import concourse.mybir as mybir
    import concourse.tile as tile
    from concourse import bacc

    f32 = mybir.dt.float32
    bf16 = mybir.dt.bfloat16

    nc = bacc.Bacc("TRN2", target_bir_lowering=False, debug=False,
                   num_devices=NCORES)

    x_in = nc.declare_dram_parameter("x", [ROWS, C], f32, isOutput=False)
    m5t_in = nc.declare_dram_parameter("m5t", [TROWS, 128], bf16, isOutput=False)
    w_in = nc.declare_dram_parameter("w", [C, C], bf16, isOutput=False)
    b_in = nc.declare_dram_parameter("bias", [TROWS, 4, C], f32, isOutput=False)
    out_ext = nc.declare_dram_parameter("out", [ROWS, C], f32, isOutput=True)

    with tile.TileContext(nc) as tc:
        with (
            tc.tile_pool(name="consts", bufs=1) as consts,
            tc.tile_pool(name="xp", bufs=3) as xp,
            tc.tile_pool(name="op", bufs=3) as op_pool,
            tc.tile_pool(name="t1s", bufs=3) as t1sp,
            tc.tile_pool(name="t1psum", bufs=2, space=tile.bass.MemorySpace.PSUM) as t1pp,
            tc.tile_pool(name="opsum", bufs=2, space=tile.bass.MemorySpace.PSUM) as opp,
        ):
            m5t_sb = consts.tile([TROWS, 128], bf16)
            w_sb = consts.tile([C, C], bf16)
            bias_sb = consts.tile([TROWS, 4, C], f32)
            nc.sync.dma_start(out=m5t_sb[:], in_=m5t_in[:])
            nc.sync.dma_start(out=w_sb[:], in_=w_in[:])
            nc.sync.dma_start(out=bias_sb[:], in_=b_in[:])

            # groups: (row_start, [(chunk_idx, nrows), ...])
            groups = []
            for g in range(FULL_GROUPS):
                groups.append((g * NCH * TROWS, [(k, TROWS) for k in range(NCH)]))
            groups.append((FULL_GROUPS * NCH * TROWS,
                           [(k, TROWS) for k in range(12)] + [(12, 100)]))

            for row0, chunks in groups:
                nfull = sum(1 for _, nr in chunks if nr == TROWS)
                has_rem = len(chunks) != nfull

                x_t = xp.tile([128, NCH, C], bf16, tag="x")
                src = x_in[row0:row0 + nfull * TROWS, :].rearrange(
                    "(k p) c -> p k c", p=TROWS)
                # f32 -> bf16 cast happens in the DMA datapath (SWDGE)
                nc.gpsimd.dma_start(out=x_t[0:TROWS, 0:nfull, :], in_=src)
                if has_rem:
                    nc.gpsimd.dma_start(
                        out=x_t[0:100, nfull, :],
                        in_=x_in[row0 + nfull * TROWS:row0 + nfull * TROWS + 100, :])

                o_t = op_pool.tile([128, NCH, C], f32, tag="o")

                for j0 in range(0, len(chunks), 4):
                    batch = chunks[j0:j0 + 4]
                    nb = len(batch)
                    t1p = t1pp.tile([128, 4, C], f32, tag="t1p")
                    for u, (k, nr) in enumerate(batch):
                        nc.tensor.matmul(t1p[:, u, :],
                                         lhsT=x_t[0:nr, k, :],
                                         rhs=m5t_sb[0:nr, :],
                                         start=True, stop=True)
                    t1s = t1sp.tile([128, 4, C], bf16, tag="t1s")
                    nc.scalar.copy(t1s[:, 0:nb, :], t1p[:, 0:nb, :])
                    o_ps = opp.tile([128, 4, C], f32, tag="ops")
                    for u, (k, nr) in enumerate(batch):
                        nc.tensor.matmul(o_ps[:, u, :],
                                         lhsT=t1s[:, u, :],
                                         rhs=w_sb[:, :],
                                         start=True, stop=True)
                    vr = min(nr for _, nr in batch)  # 125, or 100 for the tail
                    nc.vector.tensor_add(o_t[0:vr, j0:j0 + nb, :],
                                         o_ps[0:vr, 0:nb, :],
                                         bias_sb[0:vr, 0:nb, :])

                dst = out_ext[row0:row0 + nfull * TROWS, :].rearrange(
                    "(k p) c -> p k c", p=TROWS)
                nc.sync.dma_start(out=dst, in_=o_t[0:TROWS, 0:nfull, :])
                if has_rem:
                    nc.sync.dma_start(
                        out=out_ext[row0 + nfull * TROWS:row0 + nfull * TROWS + 100, :],
                        in_=o_t[0:100, nfull, :])

    nc.compile()
    return nc


def _get_graph():
    if "nc" not in _CACHE:
        _CACHE["nc"] = _build_graph()
    return _CACHE["nc"]


def _host_prep(edge_index, gcn_W, gcn_b):
    ei = np.asarray(edge_index).astype(np.int64)
    rows, cols = ei[0], ei[1]
    deg = np.bincount(cols, minlength=V).astype(np.float32) + 1.0  # + self loop
    dis = (1.0 / np.sqrt(deg)).astype(np.float32)
    M = np.zeros((V, V), np.float32)
    np.add.at(M, (cols, rows), dis[rows] * dis[cols])
    M[np.arange(V), np.arange(V)] += dis * dis
    m5t = np.zeros((TROWS, 128), np.float32)
    m5t[:, :TROWS] = np.kron(np.eye(FPT, dtype=np.float32), M.T)
    m5t_bf = m5t.astype(ml_dtypes.bfloat16)
    w_bf = np.asarray(gcn_W, np.float32).astype(ml_dtypes.bfloat16)
    bias_t = np.ascontiguousarray(
        np.broadcast_to(np.asarray(gcn_b, np.float32), (TROWS, 4, C)))
    return m5t_bf, w_bf, bias_t


def kernel(x, edge_index, adj_matrix=None, aw_W=None, aw_b=None,
           gcn_W=None, gcn_b=None, **_unused):
    from concourse.bass_utils import run_bass_kernel_spmd

    x = np.asarray(x, np.float32)
    m5t_bf, w_bf, bias_t = _host_prep(edge_index, gcn_W, gcn_b)
    xs = x.reshape(NCORES, ROWS, C)
    in_maps = [{"x": xs[i], "m5t": m5t_bf, "w": w_bf, "bias": bias_t}
               for i in range(NCORES)]
    res = run_bass_kernel_spmd(_get_graph(), in_maps,
                               core_ids=list(range(NCORES)))
    out = np.stack([r["out"] for r in res.results])
    return out.reshape(B, V, C)


# revision 3
# speedup vs baseline: 1.2505x; 1.2505x over previous
"""AdaptiveGCNLayer Trainium2 kernel (8 NeuronCores, data-parallel over frames).

The reference module's adaptive-adjacency branch is dead code (its result is
never used).  Because edge_index is shared by every frame (offsets just shift
it per frame), the live computation collapses to

    out[f] = M @ x[f] @ gcn_W + gcn_b        for every frame f

with a single 25x25 normalized-adjacency matrix M (PyG GCNConv norm with
self-loops) computed on host from the 48 edges.

Sharding: frames are data-parallel across the 8 cores.  Each core's shard is
packed on host into tile-major layout [125 partitions, 205 tiles, 128 ch]
(5 frames = 125 rows per tile; the ragged tail is zero-padded) so every
HBM<->SBUF DMA is per-partition contiguous.

Device kernel (per core):
  - mm1: T1 = lhsT(x_tile).T @ (I5 (x) M^T)   -> (M5 @ X)^T in PSUM (no transposes)
  - ACT copy+cast T1 -> SBUF bf16
  - mm2: O = lhsT(T1).T @ W                   -> natural row-major output in PSUM
  - DVE adds bias while copying PSUM -> SBUF
  - big contiguous DMAs: input on gpsimd (SWDGE, f32->bf16 cast in the DMA
    datapath), output on sync (HWDGE)
"""

import numpy as np
import ml_dtypes

B, V, C = 8192, 25, 128
NCORES = 8
FRAMES_PER_CORE = B // NCORES          # 1024
ROWS = FRAMES_PER_CORE * V             # 25600
FPT = 5                                # frames per matmul tile
TROWS = FPT * V                        # 125 rows per tile
NT = 205                               # tiles per core (last one padded)
FULL_T = ROWS // TROWS                 # 204 full tiles
TAIL_ROWS = ROWS - FULL_T * TROWS      # 100
TPG = 41                               # tiles per DMA group
NGROUPS = NT // TPG                    # 5
JB = 4                                 # tiles per PSUM batch

_CACHE = {}


def _build_graph():
    import concourse.mybir as mybir
    import concourse.tile as tile
    from concourse import bacc

    f32 = mybir.dt.float32
    bf16 = mybir.dt.bfloat16

    nc = bacc.Bacc("TRN2", target_bir_lowering=False, debug=False,
                   num_devices=NCORES)

    x_in = nc.declare_dram_parameter("x", [TROWS, NT, C], f32, isOutput=False)
    m5t_in = nc.declare_dram_parameter("m5t", [TROWS, C], bf16, isOutput=False)
    w_in = nc.declare_dram_parameter("w", [C, C], bf16, isOutput=False)
    b_in = nc.declare_dram_parameter("bias", [TROWS, JB, C], f32, isOutput=False)
    out_ext = nc.declare_dram_parameter("out", [TROWS, NT, C], f32, isOutput=True)

    with tile.TileContext(nc) as tc:
        with (
            tc.tile_pool(name="consts", bufs=1) as consts,
            tc.tile_pool(name="xp", bufs=3) as xp,
            tc.tile_pool(name="op", bufs=2) as op_pool,
            tc.tile_pool(name="t1s", bufs=3) as t1sp,
            tc.tile_pool(name="t1psum", bufs=2, space=tile.bass.MemorySpace.PSUM) as t1pp,
            tc.tile_pool(name="opsum", bufs=2, space=tile.bass.MemorySpace.PSUM) as opp,
        ):
            m5t_sb = consts.tile([TROWS, C], bf16)
            w_sb = consts.tile([C, C], bf16)
            bias_sb = consts.tile([TROWS, JB, C], f32)
            nc.sync.dma_start(out=m5t_sb[:], in_=m5t_in[:])
            nc.sync.dma_start(out=w_sb[:], in_=w_in[:])
            nc.sync.dma_start(out=bias_sb[:], in_=b_in[:])

            for g in range(NGROUPS):
                t0 = g * TPG
                x_t = xp.tile([128, TPG, C], bf16, tag="x")
                # f32 -> bf16 cast happens in the DMA datapath (SWDGE);
                # both sides are per-partition contiguous.
                nc.gpsimd.dma_start(out=x_t[0:TROWS, :, :],
                                    in_=x_in[:, t0:t0 + TPG, :])

                o_t = op_pool.tile([128, TPG, C], f32, tag="o")

                for j0 in range(0, TPG, JB):
                    nb = min(JB, TPG - j0)
                    t1p = t1pp.tile([128, JB, C], f32, tag="t1p")
                    for u in range(nb):
                        nc.tensor.matmul(t1p[:, u, :],
                                         lhsT=x_t[0:TROWS, j0 + u, :],
                                         rhs=m5t_sb[:, :],
                                         start=True, stop=True)
                    t1s = t1sp.tile([128, JB, C], bf16, tag="t1s")
                    nc.scalar.copy(t1s[:, 0:nb, :], t1p[:, 0:nb, :])
                    o_ps = opp.tile([128, JB, C], f32, tag="ops")
                    for u in range(nb):
                        nc.tensor.matmul(o_ps[:, u, :],
                                         lhsT=t1s[:, u, :],
                                         rhs=w_sb[:, :],
                                         start=True, stop=True)
                    nc.vector.tensor_add(o_t[0:TROWS, j0:j0 + nb, :],
                                         o_ps[0:TROWS, 0:nb, :],
                                         bias_sb[:, 0:nb, :])

                nc.sync.dma_start(out=out_ext[:, t0:t0 + TPG, :],
                                  in_=o_t[0:TROWS, :, :])

    nc.compile()
    return nc


def _get_graph():
    if "nc" not in _CACHE:
        _CACHE["nc"] = _build_graph()
    return _CACHE["nc"]


def _host_prep(edge_index, gcn_W, gcn_b):
    ei = np.asarray(edge_index).astype(np.int64)
    rows, cols = ei[0], ei[1]
    deg = np.bincount(cols, minlength=V).astype(np.float32) + 1.0  # + self loop
    dis = (1.0 / np.sqrt(deg)).astype(np.float32)
    M = np.zeros((V, V), np.float32)
    np.add.at(M, (cols, rows), dis[rows] * dis[cols])
    M[np.arange(V), np.arange(V)] += dis * dis
    m5t = np.kron(np.eye(FPT, dtype=np.float32), M.T)  # (125, 125)
    m5t_pad = np.zeros((TROWS, C), np.float32)
    m5t_pad[:, :TROWS] = m5t
    m5t_bf = m5t_pad.astype(ml_dtypes.bfloat16)
    w_bf = np.asarray(gcn_W, np.float32).astype(ml_dtypes.bfloat16)
    bias_t = np.ascontiguousarray(
        np.broadcast_to(np.asarray(gcn_b, np.float32), (TROWS, JB, C)))
    return m5t_bf, w_bf, bias_t


def _pack(x):
    """(B, V, C) f32 -> per-core tile-major [NCORES, TROWS, NT, C]."""
    xr = np.asarray(x, np.float32).reshape(NCORES, ROWS, C)
    packed = np.zeros((NCORES, NT, TROWS, C), np.float32)
    packed[:, :FULL_T] = xr[:, :FULL_T * TROWS].reshape(NCORES, FULL_T, TROWS, C)
    packed[:, FULL_T, :TAIL_ROWS] = xr[:, FULL_T * TROWS:]
    return np.ascontiguousarray(packed.transpose(0, 2, 1, 3))


def _unpack(outs):
    """[NCORES, TROWS, NT, C] -> (B, V, C)."""
    o = outs.transpose(0, 2, 1, 3)  # [NCORES, NT, TROWS, C]
    res = np.empty((NCORES, ROWS, C), np.float32)
    res[:, :FULL_T * TROWS] = o[:, :FULL_T].reshape(NCORES, FULL_T * TROWS, C)
    res[:, FULL_T * TROWS:] = o[:, FULL_T, :TAIL_ROWS]
    return res.reshape(B, V, C)


def kernel(x, edge_index, adj_matrix=None, aw_W=None, aw_b=None,
           gcn_W=None, gcn_b=None, **_unused):
    from concourse.bass_utils import run_bass_kernel_spmd

    m5t_bf, w_bf, bias_t = _host_prep(edge_index, gcn_W, gcn_b)
    xp = _pack(x)
    in_maps = [{"x": xp[i], "m5t": m5t_bf, "w": w_bf, "bias": bias_t}
               for i in range(NCORES)]
    res = run_bass_kernel_spmd(_get_graph(), in_maps,
                               core_ids=list(range(NCORES)))
    out = np.stack([r["out"] for r in res.results])
    return _unpack(out)
